# revision 7
# baseline (speedup 1.0000x reference)
"""Trainium2 Bass kernel for nn_DRUCell: 8-way data-parallel DRU cell.

reference:
    xh = concat([x, h], 1)                  # [B, IN+H]
    lin = xh @ W_in + b_in                  # [B, 2H]
    learn = tanh(lin[:, :H]); f = sigmoid(lin[:, H:])
    h_new = f * h + (1 - f) * learn
    out = tanh(concat([x, h_new], 1) @ W_out + b_out)
    returns (out, h_new)

Strategy: shard batch across the 8 NeuronCores (2048 rows each), replicate
weights. On-device everything lives feature-major ([feature, batch]) so the
TensorE contraction (over features) maps to partitions with no on-device
transposes; the host pre-transposes the shards (free relative to HW time) and
transposes the outputs back. Matmul operands run in bf16 (fp32 PSUM
accumulation); h is kept in fp32 for the elementwise h_new update.

Schedule notes (from NTFF profiles):
- DMA issue is split across the two HWDGE rings: SP carries the tile-0
  critical path (x0, h0, W_in) + stores, ACT carries the rest.
- A short stream of dummy matmuls runs during the load phase to warm the PE
  HAM clock gate, so the real matmuls start at 2.4 GHz.
- mm1 runs each (learn_c, forget_c) pair k-outer so every arriving W_in
  chunk unlocks two matmuls immediately (dense PE during the load trickle),
  and the h_new DVE chain for chunk c overlaps the remaining pairs.
- mm2 runs k-outer into one 4-bank PSUM tile; its x-part hides the h_new
  bf16 cast chain, and two [128,1024]-wide activations + two stores (one per
  HWDGE ring) shorten the kernel tail.
- Stores stay on HWDGE; the GpSimd SWDGE path hard-hangs the device with
  these 3D access patterns.
"""

import numpy as np
import ml_dtypes
from contextlib import ExitStack

import concourse.bass as bass
import concourse.mybir as mybir
import concourse.tile as tile
from concourse import bacc
from concourse.bass_utils import run_bass_kernel_spmd

B, IN, H = 16384, 512, 512
NCORES = 8
BL = B // NCORES  # batch rows per core
P = 128
NB = 512          # batch columns per device tile
NT = BL // NB
KIN = IN // P     # x feature chunks
KH = H // P       # h feature chunks
K1 = KIN + KH     # contraction chunks for both matmuls
MO1 = 2 * H // P  # mm1 output chunks (learn 0..KH-1, forget KH..)
MO2 = H // P      # mm2 output chunks
N_WARMUP = 14     # dummy matmuls to warm the PE HAM gate

MM_MODE = "bf16"  # "bf16" | "f32r" | "f32"

_nc_cache = {}


def _build(mm_mode):
    f32 = mybir.dt.float32
    bf16 = mybir.dt.bfloat16
    mm_dt = bf16 if mm_mode == "bf16" else f32

    def mm_ap(ap):
        return ap.bitcast(mybir.dt.float32r) if mm_mode == "f32r" else ap

    nc = bacc.Bacc("TRN2", target_bir_lowering=False, debug=False, num_devices=NCORES)

    xT_d = nc.dram_tensor("xT", [IN, BL], mm_dt, kind="ExternalInput")
    hT_d = nc.dram_tensor("hT", [H, BL], f32, kind="ExternalInput")
    w_in_d = nc.dram_tensor("w_in", [IN + H, 2 * H], mm_dt, kind="ExternalInput")
    w_out_d = nc.dram_tensor("w_out", [IN + H, H], mm_dt, kind="ExternalInput")
    b_in_d = nc.dram_tensor("b_in", [P, MO1], f32, kind="ExternalInput")
    b_out_d = nc.dram_tensor("b_out", [P, MO2], f32, kind="ExternalInput")
    h_newT_d = nc.dram_tensor("h_newT", [H, BL], f32, kind="ExternalOutput")
    outT_d = nc.dram_tensor("outT", [H, BL], f32, kind="ExternalOutput")

    AFT = mybir.ActivationFunctionType

    # feature-major DRAM views: row (c*128 + p) <-> (partition p, chunk c)
    x_dram = xT_d.ap().rearrange("(c p) n -> p c n", p=P)
    h_dram = hT_d.ap().rearrange("(c p) n -> p c n", p=P)
    w_in_dram = w_in_d.ap().rearrange("(k p) m -> p k m", p=P)
    hn_dram = h_newT_d.ap().rearrange("(c p) n -> p c n", p=P)
    out_dram = outT_d.ap().rearrange("(c p) n -> p c n", p=P)
    w_out_dram = w_out_d.ap().rearrange("(k p) m -> p k m", p=P)

    with tile.TileContext(nc) as tc, ExitStack() as ctx:
        cpool = ctx.enter_context(tc.tile_pool(name="consts", bufs=1))
        work = ctx.enter_context(tc.tile_pool(name="work", bufs=2))
        tmp_pool = ctx.enter_context(tc.tile_pool(name="tmp", bufs=4))
        psum1 = ctx.enter_context(tc.tile_pool(name="psum1", bufs=4, space="PSUM"))
        psum2 = ctx.enter_context(tc.tile_pool(name="psum2", bufs=1, space="PSUM"))

        x_sb = [cpool.tile([P, KIN * NB], mm_dt, name=f"x_sb_{j}") for j in range(NT)]
        h_sb = [cpool.tile([P, KH * NB], f32, name=f"h_sb_{j}") for j in range(NT)]

        def load_x(eng, j):
            bs = bass.ts(j, NB)
            eng.dma_start(
                x_sb[j][:].rearrange("p (k n) -> p k n", k=KIN), x_dram[:, :, bs]
            )

        def load_h(eng, j, lo, hi):
            bs = bass.ts(j, NB)
            hv = h_sb[j][:].rearrange("p (c n) -> p c n", c=KH)
            eng.dma_start(hv[:, lo:hi, :], h_dram[:, lo:hi, bs])

        # SP ring: the tile-0 critical path, interleaved so the PE's
        # (pair-k-outer) matmul stream stays fed as chunks land
        w_in_sb = [cpool.tile([P, 2 * H], mm_dt, name=f"w_in_{k}") for k in range(K1)]
        load_x(nc.sync, 0)
        load_h(nc.sync, 0, 0, KH // 2)
        nc.sync.dma_start(w_in_sb[0][:], w_in_dram[:, 0, :])
        nc.sync.dma_start(w_in_sb[1][:], w_in_dram[:, 1, :])
        load_h(nc.sync, 0, KH // 2, KH)
        for k in range(2, K1):
            nc.sync.dma_start(w_in_sb[k][:], w_in_dram[:, k, :])

        # ACT ring: everything not needed in the first ~15us
        b_in_sb = cpool.tile([P, MO1], f32, name="b_in_sb")
        nc.scalar.dma_start(b_in_sb[:], b_in_d[:])
        b_out_sb = cpool.tile([P, MO2], f32, name="b_out_sb")
        nc.scalar.dma_start(b_out_sb[:], b_out_d[:])
        load_x(nc.scalar, 1)
        load_h(nc.scalar, 1, 0, KH)
        w_out_sb = cpool.tile([P, K1 * H], mm_dt, name="w_out_sb")
        nc.scalar.dma_start(
            w_out_sb[:].rearrange("p (k m) -> p k m", k=K1), w_out_dram[:]
        )
        for j in range(2, NT):
            load_x(nc.scalar, j)
            load_h(nc.scalar, j, 0, KH)

        # ---- PE warm-up: dummy matmuls on a memset tile while loads run ----
        warm_src = cpool.tile([P, NB], bf16, name="warm_src")
        nc.gpsimd.memset(warm_src[:], 0.0)
        for w in range(N_WARMUP):
            wps = psum1.tile([P, NB], f32, name="warm_ps", tag="ps1")
            nc.tensor.matmul(
                wps[:], warm_src[:, 0:P], warm_src[:], start=True, stop=True
            )

        for j in range(NT):
            bs = bass.ts(j, NB)

            if mm_mode == "bf16":
                hc = work.tile([P, KH * NB], bf16, name="hc", tag="hc")
                for c in range(KH):
                    cs = bass.ts(c, NB)
                    nc.vector.tensor_copy(hc[:, cs], h_sb[j][:, cs])
            else:
                hc = h_sb[j]

            learn = work.tile([P, KH * NB], f32, name="learn", tag="learn")
            forget = work.tile([P, KH * NB], f32, name="forget", tag="forget")
            hn = work.tile([P, KH * NB], f32, name="hn", tag="hn")
            if mm_mode == "bf16":
                hnc = work.tile([P, KH * NB], bf16, name="hnc", tag="hnc")

            # mm1: (learn_c, forget_c) pairs, k-outer within the pair, so the
            # DVE h_new chain for chunk c overlaps the remaining pairs and
            # each W_in chunk unlocks work the moment it lands
            for c in range(KH):
                ps_l = psum1.tile([P, NB], f32, name="ps_l", tag="ps1")
                ps_f = psum1.tile([P, NB], f32, name="ps_f", tag="ps1")
                for k in range(K1):
                    rhs = (
                        x_sb[j][:, bass.ts(k, NB)]
                        if k < KIN
                        else hc[:, bass.ts(k - KIN, NB)]
                    )
                    for ps, mo in ((ps_l, c), (ps_f, c + KH)):
                        nc.tensor.matmul(
                            ps[:],
                            mm_ap(w_in_sb[k][:, mo * P:(mo + 1) * P]),
                            mm_ap(rhs),
                            start=(k == 0),
                            stop=(k == K1 - 1),
                        )
                cs = bass.ts(c, NB)
                nc.scalar.activation(
                    learn[:, cs], ps_l[:], AFT.Tanh, bias=b_in_sb[:, c:c + 1]
                )
                nc.scalar.activation(
                    forget[:, cs], ps_f[:], AFT.Sigmoid,
                    bias=b_in_sb[:, c + KH:c + KH + 1],
                )
                t = tmp_pool.tile([P, NB], f32, name="t", tag="t")
                nc.vector.tensor_sub(t[:], h_sb[j][:, cs], learn[:, cs])
                nc.vector.tensor_mul(t[:], t[:], forget[:, cs])
                nc.vector.tensor_add(hn[:, cs], t[:], learn[:, cs])
                if mm_mode == "bf16":
                    nc.vector.tensor_copy(hnc[:, cs], hn[:, cs])

            nc.sync.dma_start(
                hn_dram[:, :, bs], hn[:].rearrange("p (c n) -> p c n", c=KH)
            )

            # mm2 k-outer into one 4-bank PSUM tile: the x-part (k<KIN)
            # streams while the last h_new chunks are still being produced;
            # hnc chunk c is only needed at stage k = KIN + c.
            hsrc = hnc if mm_mode == "bf16" else hn
            ps2 = psum2.tile([P, MO2 * NB], f32, name="ps2", tag="ps2")
            for k in range(K1):
                rhs = (
                    x_sb[j][:, bass.ts(k, NB)]
                    if k < KIN
                    else hsrc[:, bass.ts(k - KIN, NB)]
                )
                for mo in range(MO2):
                    nc.tensor.matmul(
                        ps2[:, bass.ts(mo, NB)],
                        mm_ap(w_out_sb[:, (k * MO2 + mo) * P:(k * MO2 + mo + 1) * P]),
                        mm_ap(rhs),
                        start=(k == 0),
                        stop=(k == K1 - 1),
                    )
            out_t = work.tile([P, MO2 * NB], f32, name="out_t", tag="out_t")
            half = MO2 // 2
            for mo in range(MO2):
                nc.scalar.activation(
                    out_t[:, bass.ts(mo, NB)],
                    ps2[:, bass.ts(mo, NB)],
                    AFT.Tanh,
                    bias=b_out_sb[:, mo:mo + 1],
                )
                if mo == half - 1:
                    nc.sync.dma_start(
                        out_dram[:, 0:half, bs],
                        out_t[:, 0:half * NB].rearrange("p (c n) -> p c n", c=half),
                    )
                elif mo == MO2 - 1:
                    nc.scalar.dma_start(
                        out_dram[:, half:MO2, bs],
                        out_t[:, half * NB:].rearrange("p (c n) -> p c n", c=half),
                    )

    nc.compile()
    return nc


def _get_nc(mm_mode):
    if mm_mode not in _nc_cache:
        _nc_cache[mm_mode] = _build(mm_mode)
    return _nc_cache[mm_mode]


def _run(x, h, W_in, b_in, W_out, b_out, mm_mode=MM_MODE, trace=False):
    x = np.asarray(x, dtype=np.float32)
    h = np.asarray(h, dtype=np.float32)
    W_in = np.asarray(W_in, dtype=np.float32)
    b_in = np.asarray(b_in, dtype=np.float32)
    W_out = np.asarray(W_out, dtype=np.float32)
    b_out = np.asarray(b_out, dtype=np.float32)

    bf16 = ml_dtypes.bfloat16
    mm_np = bf16 if mm_mode == "bf16" else np.float32
    w_in_m = np.ascontiguousarray(W_in.astype(mm_np))
    w_out_m = np.ascontiguousarray(W_out.astype(mm_np))
    b_in_m = np.ascontiguousarray(b_in.reshape(MO1, P).T)
    b_out_m = np.ascontiguousarray(b_out.reshape(MO2, P).T)

    in_maps = []
    for i in range(NCORES):
        sl = slice(i * BL, (i + 1) * BL)
        m = {
            "xT": np.ascontiguousarray(x[sl].T).astype(mm_np),
            "hT": np.ascontiguousarray(h[sl].T),
            "w_in": w_in_m,
            "w_out": w_out_m,
            "b_in": b_in_m,
            "b_out": b_out_m,
        }
        in_maps.append(m)

    nc = _get_nc(mm_mode)
    res = run_bass_kernel_spmd(nc, in_maps, list(range(NCORES)), trace=trace)

    out = np.empty((B, H), dtype=np.float32)
    h_new = np.empty((B, H), dtype=np.float32)
    for i in range(NCORES):
        sl = slice(i * BL, (i + 1) * BL)
        out[sl] = res.results[i]["outT"].T
        h_new[sl] = res.results[i]["h_newT"].T
    return (out, h_new), res


def kernel(x, h, W_in, b_in, W_out, b_out):
    (out, h_new), _ = _run(x, h, W_in, b_in, W_out, b_out)
    return (out, h_new)


# revision 9
# speedup vs baseline: 1.0618x; 1.0618x over previous
"""Trainium2 Bass kernel for nn_DRUCell: 8-way data-parallel DRU cell.

reference:
    xh = concat([x, h], 1)                  # [B, IN+H]
    lin = xh @ W_in + b_in                  # [B, 2H]
    learn = tanh(lin[:, :H]); f = sigmoid(lin[:, H:])
    h_new = f * h + (1 - f) * learn
    out = tanh(concat([x, h_new], 1) @ W_out + b_out)
    returns (out, h_new)

Strategy: shard batch across the 8 NeuronCores (2048 rows each), replicate
weights. On-device everything lives feature-major ([feature, batch]) so the
TensorE contraction (over features) maps to partitions with no on-device
transposes; the host pre-transposes the shards (free relative to HW time) and
transposes the outputs back. Matmul operands run in bf16 (fp32 PSUM
accumulation); h is kept in fp32 for the elementwise h_new update.

Schedule notes (from NTFF profiles):
- DMA issue is split across the two HWDGE rings: SP carries the tile-0
  critical path (x0, h0, W_in) + stores, ACT carries the rest.
- A short stream of dummy matmuls runs during the load phase to warm the PE
  HAM clock gate, so the real matmuls start at 2.4 GHz.
- mm1 runs each (learn_c, forget_c) pair k-outer so every arriving W_in
  chunk unlocks two matmuls immediately (dense PE during the load trickle),
  and the h_new DVE chain for chunk c overlaps the remaining pairs.
- mm2 runs k-outer into one 4-bank PSUM tile; its x-part hides the h_new
  bf16 cast chain, and two [128,1024]-wide activations + two stores (one per
  HWDGE ring) shorten the kernel tail.
- Stores stay on HWDGE; the GpSimd SWDGE path hard-hangs the device with
  these 3D access patterns.
"""

import numpy as np
import ml_dtypes
from contextlib import ExitStack

import concourse.bass as bass
import concourse.mybir as mybir
import concourse.tile as tile
from concourse import bacc
from concourse.bass_utils import run_bass_kernel_spmd

B, IN, H = 16384, 512, 512
NCORES = 8
BL = B // NCORES  # batch rows per core
P = 128
NB = 512          # batch columns per device tile
NT = BL // NB
KIN = IN // P     # x feature chunks
KH = H // P       # h feature chunks
K1 = KIN + KH     # contraction chunks for both matmuls
MO1 = 2 * H // P  # mm1 output chunks (learn 0..KH-1, forget KH..)
MO2 = H // P      # mm2 output chunks
N_WARMUP = 13     # dummy matmuls to warm the PE HAM gate

MM_MODE = "bf16"  # "bf16" | "f32r" | "f32"

_nc_cache = {}


def _build(mm_mode):
    f32 = mybir.dt.float32
    bf16 = mybir.dt.bfloat16
    mm_dt = bf16 if mm_mode == "bf16" else f32

    def mm_ap(ap):
        return ap.bitcast(mybir.dt.float32r) if mm_mode == "f32r" else ap

    nc = bacc.Bacc("TRN2", target_bir_lowering=False, debug=False, num_devices=NCORES)

    xT_d = nc.dram_tensor("xT", [IN, BL], mm_dt, kind="ExternalInput")
    hT_d = nc.dram_tensor("hT", [H, BL], f32, kind="ExternalInput")
    w_in_d = nc.dram_tensor("w_in", [IN + H, 2 * H], mm_dt, kind="ExternalInput")
    w_out_d = nc.dram_tensor("w_out", [IN + H, H], mm_dt, kind="ExternalInput")
    b_in_d = nc.dram_tensor("b_in", [P, MO1], f32, kind="ExternalInput")
    b_out_d = nc.dram_tensor("b_out", [P, MO2], f32, kind="ExternalInput")
    h_newT_d = nc.dram_tensor("h_newT", [H, BL], f32, kind="ExternalOutput")
    outT_d = nc.dram_tensor("outT", [H, BL], f32, kind="ExternalOutput")

    AFT = mybir.ActivationFunctionType

    # feature-major DRAM views: row (c*128 + p) <-> (partition p, chunk c)
    x_dram = xT_d.ap().rearrange("(c p) n -> p c n", p=P)
    h_dram = hT_d.ap().rearrange("(c p) n -> p c n", p=P)
    w_in_dram = w_in_d.ap().rearrange("(k p) m -> p k m", p=P)
    hn_dram = h_newT_d.ap().rearrange("(c p) n -> p c n", p=P)
    out_dram = outT_d.ap().rearrange("(c p) n -> p c n", p=P)
    w_out_dram = w_out_d.ap().rearrange("(k p) m -> p k m", p=P)

    with tile.TileContext(nc) as tc, ExitStack() as ctx:
        cpool = ctx.enter_context(tc.tile_pool(name="consts", bufs=1))
        work = ctx.enter_context(tc.tile_pool(name="work", bufs=2))
        tmp_pool = ctx.enter_context(tc.tile_pool(name="tmp", bufs=4))
        psum1 = ctx.enter_context(tc.tile_pool(name="psum1", bufs=4, space="PSUM"))
        psum2 = ctx.enter_context(tc.tile_pool(name="psum2", bufs=1, space="PSUM"))

        x_sb = [cpool.tile([P, KIN * NB], mm_dt, name=f"x_sb_{j}") for j in range(NT)]
        h_sb = [cpool.tile([P, KH * NB], f32, name=f"h_sb_{j}") for j in range(NT)]

        def load_x(eng, j, lo, hi):
            bs = bass.ts(j, NB)
            xv = x_sb[j][:].rearrange("p (k n) -> p k n", k=KIN)
            eng.dma_start(xv[:, lo:hi, :], x_dram[:, lo:hi, bs])

        def load_h(eng, j, lo, hi):
            bs = bass.ts(j, NB)
            hv = h_sb[j][:].rearrange("p (c n) -> p c n", c=KH)
            eng.dma_start(hv[:, lo:hi, :], h_dram[:, lo:hi, bs])

        # All loads on the SP ring (HWDGE transfers are FIFO per ring, and
        # the two rings split SDMA bandwidth — one ring, criticality-ordered,
        # feeds the tile-0 critical path at full rate). Biases ride the idle
        # ACT ring since they're tiny.
        w_in_sb = [cpool.tile([P, 2 * H], mm_dt, name=f"w_in_{k}") for k in range(K1)]
        w_out_sb = [cpool.tile([P, H], mm_dt, name=f"w_out_{k}") for k in range(K1)]
        b_in_sb = cpool.tile([P, MO1], f32, name="b_in_sb")
        nc.scalar.dma_start(b_in_sb[:], b_in_d[:])
        b_out_sb = cpool.tile([P, MO2], f32, name="b_out_sb")
        nc.scalar.dma_start(b_out_sb[:], b_out_d[:])

        nc.sync.dma_start(w_in_sb[0][:], w_in_dram[:, 0, :])
        load_x(nc.sync, 0, 0, 2)
        load_h(nc.sync, 0, 0, KH // 2)
        nc.sync.dma_start(w_in_sb[1][:], w_in_dram[:, 1, :])
        load_h(nc.sync, 0, KH // 2, KH)
        load_x(nc.sync, 0, 2, KIN)
        for k in range(2, K1):
            nc.sync.dma_start(w_in_sb[k][:], w_in_dram[:, k, :])
        for k in range(K1):
            nc.sync.dma_start(w_out_sb[k][:], w_out_dram[:, k, :])
        for j in range(1, NT):
            load_x(nc.sync, j, 0, KIN)
            load_h(nc.sync, j, 0, KH)

        # ---- PE warm-up: dummy matmuls on a memset tile while loads run ----
        warm_src = cpool.tile([P, NB], bf16, name="warm_src")
        nc.gpsimd.memset(warm_src[:], 0.0)
        for w in range(N_WARMUP):
            wps = psum1.tile([P, NB], f32, name="warm_ps", tag="ps1")
            nc.tensor.matmul(
                wps[:], warm_src[:, 0:P], warm_src[:], start=True, stop=True
            )

        for j in range(NT):
            bs = bass.ts(j, NB)

            if mm_mode == "bf16":
                hc = work.tile([P, KH * NB], bf16, name="hc", tag="hc")
                for c in range(KH):
                    cs = bass.ts(c, NB)
                    nc.vector.tensor_copy(hc[:, cs], h_sb[j][:, cs])
            else:
                hc = h_sb[j]

            learn = work.tile([P, KH * NB], f32, name="learn", tag="learn")
            forget = work.tile([P, KH * NB], f32, name="forget", tag="forget")
            hn = work.tile([P, KH * NB], f32, name="hn", tag="hn")
            if mm_mode == "bf16":
                hnc = work.tile([P, KH * NB], bf16, name="hnc", tag="hnc")

            # mm1: (learn_c, forget_c) pairs, k-outer within the pair, so the
            # DVE h_new chain for chunk c overlaps the remaining pairs and
            # each W_in chunk unlocks work the moment it lands
            for c in range(KH):
                ps_l = psum1.tile([P, NB], f32, name="ps_l", tag="ps1")
                ps_f = psum1.tile([P, NB], f32, name="ps_f", tag="ps1")
                for k in range(K1):
                    rhs = (
                        x_sb[j][:, bass.ts(k, NB)]
                        if k < KIN
                        else hc[:, bass.ts(k - KIN, NB)]
                    )
                    for ps, mo in ((ps_l, c), (ps_f, c + KH)):
                        nc.tensor.matmul(
                            ps[:],
                            mm_ap(w_in_sb[k][:, mo * P:(mo + 1) * P]),
                            mm_ap(rhs),
                            start=(k == 0),
                            stop=(k == K1 - 1),
                        )
                cs = bass.ts(c, NB)
                nc.scalar.activation(
                    learn[:, cs], ps_l[:], AFT.Tanh, bias=b_in_sb[:, c:c + 1]
                )
                nc.scalar.activation(
                    forget[:, cs], ps_f[:], AFT.Sigmoid,
                    bias=b_in_sb[:, c + KH:c + KH + 1],
                )
                t = tmp_pool.tile([P, NB], f32, name="t", tag="t")
                nc.vector.tensor_sub(t[:], h_sb[j][:, cs], learn[:, cs])
                nc.vector.tensor_mul(t[:], t[:], forget[:, cs])
                nc.vector.tensor_add(hn[:, cs], t[:], learn[:, cs])
                if mm_mode == "bf16":
                    nc.vector.tensor_copy(hnc[:, cs], hn[:, cs])

            nc.sync.dma_start(
                hn_dram[:, :, bs], hn[:].rearrange("p (c n) -> p c n", c=KH)
            )

            # mm2 k-outer into one 4-bank PSUM tile: the x-part (k<KIN)
            # streams while the last h_new chunks are still being produced;
            # hnc chunk c is only needed at stage k = KIN + c.
            hsrc = hnc if mm_mode == "bf16" else hn
            ps2 = psum2.tile([P, MO2 * NB], f32, name="ps2", tag="ps2")
            for k in range(K1):
                rhs = (
                    x_sb[j][:, bass.ts(k, NB)]
                    if k < KIN
                    else hsrc[:, bass.ts(k - KIN, NB)]
                )
                for mo in range(MO2):
                    nc.tensor.matmul(
                        ps2[:, bass.ts(mo, NB)],
                        mm_ap(w_out_sb[k][:, mo * P:(mo + 1) * P]),
                        mm_ap(rhs),
                        start=(k == 0),
                        stop=(k == K1 - 1),
                    )
            out_t = work.tile([P, MO2 * NB], f32, name="out_t", tag="out_t")
            half = MO2 // 2
            for mo in range(MO2):
                nc.scalar.activation(
                    out_t[:, bass.ts(mo, NB)],
                    ps2[:, bass.ts(mo, NB)],
                    AFT.Tanh,
                    bias=b_out_sb[:, mo:mo + 1],
                )
                if mo == half - 1:
                    nc.sync.dma_start(
                        out_dram[:, 0:half, bs],
                        out_t[:, 0:half * NB].rearrange("p (c n) -> p c n", c=half),
                    )
                elif mo >= half:
                    nc.sync.dma_start(
                        out_dram[:, mo:mo + 1, bs],
                        out_t[:, bass.ts(mo, NB)].rearrange("p (c n) -> p c n", c=1),
                    )

    nc.compile()
    return nc


def _get_nc(mm_mode):
    if mm_mode not in _nc_cache:
        _nc_cache[mm_mode] = _build(mm_mode)
    return _nc_cache[mm_mode]


def _run(x, h, W_in, b_in, W_out, b_out, mm_mode=MM_MODE, trace=False):
    x = np.asarray(x, dtype=np.float32)
    h = np.asarray(h, dtype=np.float32)
    W_in = np.asarray(W_in, dtype=np.float32)
    b_in = np.asarray(b_in, dtype=np.float32)
    W_out = np.asarray(W_out, dtype=np.float32)
    b_out = np.asarray(b_out, dtype=np.float32)

    bf16 = ml_dtypes.bfloat16
    mm_np = bf16 if mm_mode == "bf16" else np.float32
    w_in_m = np.ascontiguousarray(W_in.astype(mm_np))
    w_out_m = np.ascontiguousarray(W_out.astype(mm_np))
    b_in_m = np.ascontiguousarray(b_in.reshape(MO1, P).T)
    b_out_m = np.ascontiguousarray(b_out.reshape(MO2, P).T)

    in_maps = []
    for i in range(NCORES):
        sl = slice(i * BL, (i + 1) * BL)
        m = {
            "xT": np.ascontiguousarray(x[sl].T).astype(mm_np),
            "hT": np.ascontiguousarray(h[sl].T),
            "w_in": w_in_m,
            "w_out": w_out_m,
            "b_in": b_in_m,
            "b_out": b_out_m,
        }
        in_maps.append(m)

    nc = _get_nc(mm_mode)
    res = run_bass_kernel_spmd(nc, in_maps, list(range(NCORES)), trace=trace)

    out = np.empty((B, H), dtype=np.float32)
    h_new = np.empty((B, H), dtype=np.float32)
    for i in range(NCORES):
        sl = slice(i * BL, (i + 1) * BL)
        out[sl] = res.results[i]["outT"].T
        h_new[sl] = res.results[i]["h_newT"].T
    return (out, h_new), res


def kernel(x, h, W_in, b_in, W_out, b_out):
    (out, h_new), _ = _run(x, h, W_in, b_in, W_out, b_out)
    return (out, h_new)
